# revision 4
# baseline (speedup 1.0000x reference)
"""KV page-cache scatter update on 8 Trainium2 NeuronCores.

Semantics (matches the reference):
    kv_ev = interleave(new_k, new_v)          # [T, 2H, D], head axis k0,v0,k1,v1,...
    for i in range(K):
        kv_pages[t_pages[i], t_slots[i]] = kv_ev[i]
    return kv_pages

Sharding: kv_pages is split along the page axis across the 8 cores
(256 pages each).  The host partitions the valid tokens by destination
(core, shard-half) and hands each core a compacted, interleaved update block
plus flat destination row indices relative to its half.  Each core:
  1. copies its 33.5MB page shard input -> output with large DRAM->DRAM DMAs
     (64KB descriptors, alternating across the two HWDGE rings)
  2. scatters its update rows into the output with indirect (SWDGE) DMAs
     using the destination row indices.
The output shard is split into TWO DRAM tensors (halves) so that each half's
scatter only depends on that half's bulk copy: Tile's range-based dependency
tracking then overlaps half A's scatter with half B's copy, and the two
scatters (disjoint tensors) don't serialize against each other.
Destinations are unique (page,slot) pairs, so padding duplicates the last
valid row (identical concurrent writes are benign).
"""

import numpy as np

from concourse import bacc, bass, mybir, tile
from concourse.bass_utils import run_bass_kernel_spmd

# Problem geometry (hardcoded per contract).
P, S, HH, D = 2048, 16, 16, 128   # pages, slots/page, 2*kv_heads, head_dim
T = 2048                          # new tokens
NCORES = 8
PC = P // NCORES                  # pages per core
RC = PC * S                       # flat rows per core (4096)
HR = RC // 2                      # rows per half (2048)
RD = HH * D                       # row width in f32 (2048 = 8KB)

_PROGRAM_CACHE: dict[int, object] = {}
_LAST_IN_MAPS: list | None = None  # stashed for test.py's traced re-run


def _build_program(nph: int, reps: int = 1):
    """Bass program: copy kv shard in->out (two halves), scatter nph update
    rows per half.

    reps > 1 repeats the identical body inside one NEFF (each rep re-copies
    and re-scatters, serialized by Tile's dependency tracking on the output
    halves) so a (t_repsR - t_reps1)/(R-1) slope cancels dispatch overhead.
    """
    nc = bacc.Bacc("TRN2", target_bir_lowering=False, debug=False)

    kv_in = nc.dram_tensor("kv_in", [RC, RD], mybir.dt.float32, kind="ExternalInput")
    upd = nc.dram_tensor("upd", [2 * nph, RD], mybir.dt.float32, kind="ExternalInput")
    dest = nc.dram_tensor("dest", [2 * nph, 1], mybir.dt.int32, kind="ExternalInput")
    outs = [
        nc.dram_tensor("kv_outA", [HR, RD], mybir.dt.float32, kind="ExternalOutput"),
        nc.dram_tensor("kv_outB", [HR, RD], mybir.dt.float32, kind="ExternalOutput"),
    ]

    # inner descriptor rows of 16384 f32 (64KB, the AP last-dim limit).  Under
    # low HBM contention this measured ~1.6x faster than 32KB descriptors
    # (103us vs 166us per 32MB shard copy); under heavy co-tenant load both
    # are HBM-share-bound and equal.  Larger values regress (AP splitting).
    inner = 16384
    half_elems = HR * RD
    n_chunks_half = 4
    chunk = half_elems // n_chunks_half
    chunk_rows = chunk // inner
    nb = nph // 128

    with tile.TileContext(nc) as tc:
        with tc.tile_pool(name="sbuf", bufs=max(2, 4 * nb)) as pool:
            for _rep in range(reps):
                # stage all update rows + dest indices into SBUF first; issued
                # on gpsimd (SWDGE) so they overlap the copy without occupying
                # the HWDGE rings that stream the bulk chunks
                blocks = [[], []]
                for h in range(2):
                    for b in range(nb):
                        off = h * nph + b * 128
                        utile = pool.tile([128, RD], mybir.dt.float32)
                        dtile = pool.tile([128, 1], mybir.dt.int32)
                        nc.gpsimd.dma_start(out=utile[:], in_=upd[off:off + 128, :])
                        nc.gpsimd.dma_start(out=dtile[:], in_=dest[off:off + 128, :])
                        blocks[h].append((utile, dtile))

                # half A copies first on both HWDGE rings, then half B, so
                # half A's scatter overlaps half B's bulk copy
                for h in range(2):
                    for c in range(n_chunks_half):
                        src = bass.AP(kv_in, h * half_elems + c * chunk,
                                      [[inner, chunk_rows], [1, inner]])
                        dst = bass.AP(outs[h], c * chunk,
                                      [[inner, chunk_rows], [1, inner]])
                        eng = nc.sync if c % 2 == 0 else nc.scalar
                        eng.dma_start(out=dst, in_=src)
                    for utile, dtile in blocks[h]:
                        nc.gpsimd.indirect_dma_start(
                            out=outs[h][:],
                            out_offset=bass.IndirectOffsetOnAxis(
                                ap=dtile[:, :1], axis=0),
                            in_=utile[:],
                            in_offset=None,
                        )

    nc.compile()
    return nc


def kernel(kv_pages, t_pages, t_slots, new_k, new_v, K):
    kv_pages = np.asarray(kv_pages)
    t_pages = np.asarray(t_pages)
    t_slots = np.asarray(t_slots)
    new_k = np.asarray(new_k)
    new_v = np.asarray(new_v)
    k_valid = int(np.asarray(K))

    out_dtype = kv_pages.dtype
    Tn, Hn, Dn = new_k.shape

    # interleave K/V along the head axis: [T, 2H, D] -> flat [T, RD]
    kv_ev = np.empty((Tn, 2 * Hn, Dn), dtype=out_dtype)
    kv_ev[:, 0::2, :] = new_k
    kv_ev[:, 1::2, :] = new_v
    kv_ev = kv_ev.reshape(Tn, 2 * Hn * Dn)

    rows_abs = (t_pages[:k_valid].astype(np.int64) * S
                + t_slots[:k_valid].astype(np.int64))
    core_of = rows_abs // RC
    kv_flat = kv_pages.reshape(P * S, RD)

    # group updates by (core, shard-half)
    sel = {}
    maxn = 0
    for c in range(NCORES):
        m = core_of == c
        rel = rows_abs[m] - c * RC
        gi = np.nonzero(m)[0]
        for h in range(2):
            hm = (rel // HR) == h
            sel[(c, h)] = (gi[hm], rel[hm] - h * HR)
            maxn = max(maxn, int(hm.sum()))
    nph = max(128, -(-maxn // 128) * 128)

    if nph not in _PROGRAM_CACHE:
        _PROGRAM_CACHE[nph] = _build_program(nph)
    nc = _PROGRAM_CACHE[nph]

    in_maps = []
    for c in range(NCORES):
        upd = np.empty((2 * nph, RD), dtype=out_dtype)
        dest = np.empty((2 * nph, 1), dtype=np.int32)
        for h in range(2):
            gi, rel = sel[(c, h)]
            n = len(gi)
            o = h * nph
            if n > 0:
                upd[o:o + n] = kv_ev[gi]
                dest[o:o + n, 0] = rel
                upd[o + n:o + nph] = upd[o + n - 1]
                dest[o + n:o + nph, 0] = dest[o + n - 1, 0]
            else:
                # no updates for this half: rewrite its row 0 with original data
                upd[o:o + nph] = kv_flat[c * RC + h * HR]
                dest[o:o + nph, 0] = 0
        in_maps.append({
            "kv_in": np.ascontiguousarray(kv_flat[c * RC:(c + 1) * RC]),
            "upd": upd,
            "dest": dest,
        })

    global _LAST_IN_MAPS
    _LAST_IN_MAPS = in_maps
    res = run_bass_kernel_spmd(nc, in_maps, core_ids=list(range(NCORES)))
    out = np.concatenate(
        [res.results[c][t] for c in range(NCORES) for t in ("kv_outA", "kv_outB")],
        axis=0,
    ).reshape(P, S, HH, D)
    return out.astype(out_dtype, copy=False)
